# revision 41
# baseline (speedup 1.0000x reference)
"""OHEM cross-entropy loss kernel for Trainium2 (8 NeuronCores, Bass/Tile).

Math (matches reference.py):
    logp   = log_softmax(seg_logit, axis=1)          # [B,C,H,W], C=19
    x_l    = logp at label (ignore 255 -> class 0)
    prob   = exp(x_l)
    thr    = max(sort(prob.flatten())[MIN_KEPT*B], 0.7)
    loss   = mean(-x_l * (prob < thr))

Device strategy (data-parallel over B across 8 cores, one image per core):
    The loss is a global mean over pixels, so any per-core pixel permutation
    is admissible, and the 2e-2 harness tolerance admits f16 staging of the
    logits (validated ~1e-5 end to end). The host sorts each core's pixels
    by label and stages the logits as f16 (halves HBM traffic). Two further
    consequences of the sort:
      - each 4096-pixel partition row spans at most TWO distinct labels
        (class runs ~27k pixels >> 4096; host-validated, exact-host
        fallback otherwise), so with a per-row CLASS permutation (row's
        1st label -> slot 0, 2nd -> slot 1) the 19-way label gather
        collapses to ONE predicated copy keyed on a single bit-plane,
        applied directly to the raw f16 logits (x_l needs no exp/ln);
      - adjacent pixel PAIRS share a label (<= 18 mismatched pairs per
        core, ~1e-5 error), so that predicated copy runs on i32-packed
        f16 pairs at half the element count (copy_predicated is capped
        at 1 elem/cycle on DVE).

    Per 128xF-pixel chunk:
      - DMA [128, 19, F] f16 logits (per-row class permutation applied)
      - ACT: ONE exp instruction -> eb f16 (the engine bottleneck:
        1 elem/ns/partition, ~78us total)
      - DVE: in-place pairwise tree on eb -> sumexp tile; single
        predicated pair-copy resolves x_l into lb[:,0,:]
      - ACT: Ln(sumexp) -> lse;  DVE: v = x_l - lse, then accumulate
        sum(min(v,C0)) and count(v<C0)  (min(u,0) = min(v,C0)-C0, fixed
        on the host)
    Each chunk's Ln + reduction tail is ISSUED TWO CHUNKS LATE: the tile
    scheduler orders per-engine queues by simulated readiness, and with
    zero lag it wedges Ln_j (which needs tree_j) between exp_j and
    exp_{j+1}, serializing ACT on DVE every chunk. With a 2-chunk lag the
    dependency is always at least one full exp old, so the in-order ACT
    queue never stalls and exp runs back to back.

    Host combines partials, falling back to an exact host path if
    count <= MIN_KEPT*B (never for the target distribution).
"""

import numpy as np

B = 8
C = 19
H, W = 512, 1024
HW = H * W            # 524288 pixels per image/core
P = 128               # SBUF partitions
FREE = HW // P        # 4096 pixels per partition
# small head chunk cuts pipeline-fill latency, small tail chunk cuts the
# post-last-DMA drain; few big chunks minimize per-instruction overheads
CHUNKS = [192, 256, 384, 512, 640, 640, 512, 384, 320, 256]
assert sum(CHUNKS) == FREE
FMAX = max(CHUNKS)
NCHUNK = len(CHUNKS)
C0 = float(np.log(np.float32(0.7)))
MIN_KEPT = 100000
IGNORE_INDEX = 255
N_TOTAL = B * HW

_CACHE = {}


def _build_nc():
    import bass_rust as _bass_rust
    import concourse.bacc as bacc
    import concourse.mybir as mybir
    import concourse.tile as tile
    from concourse.hw_specs import get_activation_tables

    fp16 = mybir.dt.float16
    fp32 = mybir.dt.float32
    i32 = mybir.dt.int32
    u8 = mybir.dt.uint8

    class _Bacc(bacc.Bacc):
        def insert_act_table_loads(self):
            """Same as Bacc.insert_act_table_loads, but masks Exp/Ln out of
            every act-func set except natural_log_exp_and_others (list
            positions/IDs preserved), so alternating Exp/Ln activations all
            resolve to the one set that holds both -> 1 table load total."""
            has_activation = any(
                isinstance(i, mybir.InstActivation)
                for b in self.main_func.blocks
                for i in b.instructions
            )
            if not has_activation:
                return
            both = {
                mybir.ActivationFunctionType.Exp,
                mybir.ActivationFunctionType.Ln,
            }
            tables = [
                (name, fns if name == "natural_log_exp_and_others" else fns - both)
                for name, fns in get_activation_tables(self.m.arch).items()
            ]
            _bass_rust.insert_act_table_loads(self, tables)

    nc = _Bacc()
    # f16 logits, per-row class permutation pre-applied, CHUNK-BLOCKED:
    # per partition, chunk j's [C, F_j] block is contiguous, so each chunk
    # DMA is 128 large contiguous descriptors (line-rate, cheap to issue)
    logit = nc.dram_tensor("logit", [P, C * FREE], fp16, kind="ExternalInput")
    # one bit per pixel pair: 1 -> pair's label is the row's 2nd class
    bits = nc.dram_tensor("bits", [P, FREE // 2], u8, kind="ExternalInput")
    acc = nc.dram_tensor("acc", [P, 2 * NCHUNK], fp32, kind="ExternalOutput")

    with tile.TileContext(nc) as tc:
        with (
            tc.tile_pool(name="lb", bufs=3) as lb_pool,
            tc.tile_pool(name="eb", bufs=3) as eb_pool,
            tc.tile_pool(name="bits", bufs=2) as bits_pool,
            tc.tile_pool(name="se", bufs=4) as se_pool,
            tc.tile_pool(name="ls", bufs=3) as ls_pool,
            tc.tile_pool(name="pix", bufs=3) as pix_pool,
            tc.tile_pool(name="accp", bufs=1) as acc_pool,
        ):
            acc_t = acc_pool.tile([P, 2 * NCHUNK], fp32)
            bits_t = bits_pool.tile([P, FREE // 2], u8)

            def emit_ln(j, f, se):
                # lse = Ln(sumexp) on ACT. The f+1'th element is the ordering
                # sentinel written from the NEXT chunk's exp output, which
                # pins this Ln after that exp in any valid ACT order.
                lse = ls_pool.tile([P, FMAX + 1], fp16, tag="lse")
                nc.scalar.activation(
                    out=lse[:, 0 : f + 1], in_=se[:, 0 : f + 1],
                    func=mybir.ActivationFunctionType.Ln,
                )
                return lse

            def emit_tail(j, f, xl, lse):
                # v = x_l - lse; min(u,0) = min(v,C0) - C0, [u<0] = [v<C0]
                v = pix_pool.tile([P, FMAX], fp16, tag="v")
                nc.vector.tensor_tensor(
                    out=v[:, 0:f], in0=xl[:, 0:f], in1=lse[:, 0:f],
                    op=mybir.AluOpType.subtract,
                )
                scr = pix_pool.tile([P, FMAX], fp16, tag="scr")
                nc.vector.tensor_scalar(
                    out=scr[:, 0:f], in0=v[:, 0:f], scalar1=C0, scalar2=None,
                    op0=mybir.AluOpType.min, op1=mybir.AluOpType.add,
                    accum_out=acc_t[:, j : j + 1],
                )
                scr2 = pix_pool.tile([P, FMAX], fp16, tag="scr2")
                nc.vector.tensor_scalar(
                    out=scr2[:, 0:f], in0=v[:, 0:f], scalar1=C0, scalar2=None,
                    op0=mybir.AluOpType.is_lt, op1=mybir.AluOpType.add,
                    accum_out=acc_t[:, NCHUNK + j : NCHUNK + j + 1],
                )

            prev = None  # (j, F, lb, se) of the previous chunk
            off = 0
            for j, F in enumerate(CHUNKS):
                FP = F // 2
                lb = lb_pool.tile([P, C, FMAX], fp16, tag="lb")
                eb = eb_pool.tile([P, C, FMAX], fp16, tag="eb")
                if j == 0:
                    # cold first chunk: split DMA + exp into two class
                    # halves so the first exp starts once half has landed
                    nc.sync.dma_start(
                        out=lb[:, 0:10, 0:F],
                        in_=logit[:, 0 : 10 * F].rearrange(
                            "p (c f) -> p c f", c=10
                        ),
                    )
                    nc.sync.dma_start(
                        out=lb[:, 10:C, 0:F],
                        in_=logit[:, 10 * F : C * F].rearrange(
                            "p (c f) -> p c f", c=C - 10
                        ),
                    )
                    # all pair-bits in one DMA (128 contiguous 2KB descs)
                    nc.sync.dma_start(out=bits_t[:], in_=bits[:, :])
                    nc.scalar.activation(
                        out=eb[:, 0:10, 0:F], in_=lb[:, 0:10, 0:F],
                        func=mybir.ActivationFunctionType.Exp,
                    )
                    nc.scalar.activation(
                        out=eb[:, 10:C, 0:F], in_=lb[:, 10:C, 0:F],
                        func=mybir.ActivationFunctionType.Exp,
                    )
                else:
                    nc.sync.dma_start(
                        out=lb[:, :, 0:F],
                        in_=logit[:, C * off : C * (off + F)].rearrange(
                            "p (c f) -> p c f", c=C
                        ),
                    )
                    # ACT: one fat exp f16 -> f16 (reads ALL lb rows, so
                    # the merge write into lb[:,0,:] is ordered after)
                    nc.scalar.activation(
                        out=eb[:, :, 0:F], in_=lb[:, :, 0:F],
                        func=mybir.ActivationFunctionType.Exp,
                    )

                if prev is not None:
                    pj, pf, pxl, pse = prev
                    # ordering sentinel: previous chunk's Ln input gains one
                    # element produced FROM this chunk's exp output, so the
                    # scheduler cannot legally place Ln_{j-1} before exp_j
                    # (where it would stall ACT on tree_{j-1})
                    nc.vector.tensor_copy(
                        out=pse[:, pf : pf + 1], in_=eb[:, 18, 0:1]
                    )
                    plse = emit_ln(pj, pf, pse)

                # DVE: in-place sumexp tree (f16 2x)
                nc.vector.tensor_tensor(
                    out=eb[:, 0:9, 0:F], in0=eb[:, 0:9, 0:F],
                    in1=eb[:, 9:18, 0:F], op=mybir.AluOpType.add,
                )
                # single gather merge on raw f16 logit pairs: x_l -> lb[:,0,:]
                nc.vector.copy_predicated(
                    out=lb[:, 0, 0:F].bitcast(i32),
                    mask=bits_t[:, off // 2 : off // 2 + FP],
                    data=lb[:, 1, 0:F].bitcast(i32),
                )
                # evacuate x_l so lb recycles right after the merge (the DMA
                # for chunk j+3 must not wait on chunk j's reduction tail)
                xl = pix_pool.tile([P, FMAX], fp16, tag="xl")
                nc.vector.tensor_copy(out=xl[:, 0:F], in_=lb[:, 0, 0:F])
                nc.vector.tensor_tensor(
                    out=eb[:, 8, 0:F], in0=eb[:, 8, 0:F], in1=eb[:, 18, 0:F],
                    op=mybir.AluOpType.add,
                )
                nc.vector.tensor_tensor(
                    out=eb[:, 0:4, 0:F], in0=eb[:, 0:4, 0:F],
                    in1=eb[:, 4:8, 0:F], op=mybir.AluOpType.add,
                )
                nc.vector.tensor_tensor(
                    out=eb[:, 0:2, 0:F], in0=eb[:, 0:2, 0:F],
                    in1=eb[:, 2:4, 0:F], op=mybir.AluOpType.add,
                )
                nc.vector.tensor_tensor(
                    out=eb[:, 0, 0:F], in0=eb[:, 0, 0:F], in1=eb[:, 1, 0:F],
                    op=mybir.AluOpType.add,
                )
                se = se_pool.tile([P, FMAX + 1], fp16, tag="se")
                nc.vector.tensor_tensor(
                    out=se[:, 0:F], in0=eb[:, 0, 0:F], in1=eb[:, 8, 0:F],
                    op=mybir.AluOpType.add,
                )

                if prev is not None:
                    emit_tail(pj, pf, pxl, plse)
                prev = (j, F, xl, se)
                off += F

            # drain the last chunk (no sentinel: nothing follows it)
            pj, pf, pxl, pse = prev
            nc.vector.tensor_copy(out=pse[:, pf : pf + 1], in_=pse[:, 0:1])
            plse = emit_ln(pj, pf, pse)
            emit_tail(pj, pf, pxl, plse)

            nc.sync.dma_start(out=acc[:, :], in_=acc_t[:])
    nc.finalize()
    return nc


def _host_fallback(seg_logit, seg_label):
    """Exact numpy replication of the reference (quantile path included)."""
    x = np.asarray(seg_logit, dtype=np.float32)
    lbl = np.asarray(seg_label)
    Bn, Cn = x.shape[0], x.shape[1]
    xf = x.reshape(Bn, Cn, -1)
    m = xf.max(axis=1, keepdims=True)
    e = np.exp(xf - m)
    lse = np.log(e.sum(axis=1, keepdims=True)) + m
    logp = xf - lse
    l2 = np.where(lbl == IGNORE_INDEX, 0, lbl).reshape(Bn, 1, -1).astype(np.int64)
    lp_at = np.take_along_axis(logp, l2, axis=1)[:, 0]
    prob = np.exp(lp_at)
    sortp = np.sort(prob.reshape(-1))
    idx = min(MIN_KEPT * Bn, sortp.shape[0] - 1)
    thr = max(float(sortp[idx]), np.float32(0.7))
    wgt = (prob < thr).astype(np.float32)
    return np.float32((-lp_at * wgt).mean())


def _prep_core(x, lbl):
    """Sort pixels by label, renumber classes per partition row, build the
    pair bit-plane, stage logits as f16. Returns (xr [P,C,FREE] f16,
    bits [P,FREE/2] u8) or None if a row has >2 distinct labels."""
    order = np.argsort(lbl, kind="stable")
    ls = lbl[order].reshape(P, FREE)
    first = ls[:, 0]
    last = ls[:, -1]
    # sorted within each row -> distinct count = change count + 1
    if ((ls[:, 1:] != ls[:, :-1]).sum(axis=1) > 1).any():
        return None
    second = np.where(last != first, last, (first + 1) % C)
    # per-row class permutation: slot0=first, slot1=second, rest arbitrary
    pi = np.empty((P, C), dtype=np.int64)
    pi[:, 0] = first
    pi[:, 1] = second
    allc = np.arange(C)
    for p in range(P):
        rest = allc[(allc != first[p]) & (allc != second[p])]
        pi[p, 2:] = rest
    # gather with pixel-permute and class-permute fused, f16 staging
    xr = x[pi[:, :, None], order.reshape(P, 1, FREE)].astype(np.float16)
    # chunk-blocked layout: per partition, each chunk's [C, F] contiguous
    blocks = []
    o = 0
    for f in CHUNKS:
        blocks.append(xr[:, :, o : o + f].reshape(P, C * f))
        o += f
    xb = np.concatenate(blocks, axis=1)                 # [P, C*FREE]
    bitsp = (ls[:, 0::2] != first[:, None]).astype(np.uint8)  # [P, FREE/2]
    return np.ascontiguousarray(xb), np.ascontiguousarray(bitsp)


def kernel(seg_logit, seg_label):
    from concourse import bass_utils

    x = np.ascontiguousarray(np.asarray(seg_logit, dtype=np.float32)).reshape(
        B, C, HW
    )
    lbl = np.asarray(seg_label)
    lbl = np.where(lbl == IGNORE_INDEX, 0, lbl).astype(np.uint8).reshape(B, HW)

    in_maps = []
    for b in range(B):
        prep = _prep_core(x[b], lbl[b])
        if prep is None:
            return _host_fallback(seg_logit, seg_label)
        xr, bitsp = prep
        in_maps.append({"logit": xr, "bits": bitsp})

    if "nc" not in _CACHE:
        _CACHE["nc"] = _build_nc()
    nc = _CACHE["nc"]

    res = bass_utils.run_bass_kernel_spmd(nc, in_maps, core_ids=list(range(B)))

    racc = 0.0
    wacc = 0.0
    for r in res.results:
        a = r["acc"]
        racc += float(a[:, :NCHUNK].sum(dtype=np.float64))
        wacc += float(a[:, NCHUNK:].sum(dtype=np.float64))

    if wacc <= MIN_KEPT * B:
        # quantile threshold exceeds 0.7 -> exact host path (rare/never for
        # the target distribution)
        return _host_fallback(seg_logit, seg_label)

    # racc sums min(v, C0) = min(u,0) + C0 per pixel; undo the constant
    sum_min = racc - C0 * N_TOTAL
    total = -(sum_min + C0 * wacc)
    return np.float32(total / N_TOTAL)
